# revision 33
# baseline (speedup 1.0000x reference)
"""Bahdanau-attention kernel for Trainium2, data-parallel over batch on 8 NeuronCores.

Computes, per the reference:
    qf   = (query @ w_q)[0]                       # (B, 16)
    U    = tanh(ref @ w_ref + qf) @ v             # (SEQ, B)
    U    = tanh(U)
    U   += (1 - mask) * -1e10
    prob = softmax(U * 10, axis=0).T              # (B, SEQ)
    samp = argmax(prob, axis=1)                   # (B,)

Sharding: batch axis split 8 ways (256 batches/core); softmax over SEQ is local.

Per-core dataflow (B_local=256 = 2 halves x 128 partitions; groups of 8 batches
pack the 16-dim feature axis into 128 partitions as p = u*16+d):
  1. DMA ref in natural layout [128 s-rows, 256*16 free] (contiguous 16KB rows).
  2. PE transpose 128x128 blocks (one block = 8 batches x 16 dims) -> PSUM.
  3. DVE/ACT copy PSUM -> SBUF (PE cannot read PSUM).
  4. matmul vs block-diag(w_ref) in float32r (full-rate fp32) -> Y^T in PSUM.
  5. ScalarE tanh with per-partition bias = query-feature column -> SBUF.
  6. matmul vs per-group block-diag(v), accumulating U [128 b, 512 s] in PSUM.
  7. ScalarE tanh -> U in SBUF; add (1-mask^T)*-1e10 (mask^T via PE transpose).
  8. Softmax along free axis: reduce_max, Exp with fused bias/scale/accum-sum,
     reciprocal, scale; argmax via DVE max/max_index. DMA prob + sample out.
"""

import sys

import numpy as np

if "/opt/trn_rl_repo" not in sys.path:
    sys.path.insert(0, "/opt/trn_rl_repo")

SEQ = 2048
B = 2048
D = 16
NCORES = 8
BL = B // NCORES  # 256 batches per core
NH = 2  # halves of 128 batches
SCHUNK = 512  # seq positions per chunk
NSC = SEQ // SCHUNK  # 4
NG = 16  # groups of 8 batches per half
CSCALE = 10.0

_CACHE = {}


def _build_bass():
    import concourse.bass as bass
    import concourse.mybir as mybir
    import concourse.tile as tile
    from concourse import bacc, masks
    from contextlib import ExitStack

    f32 = mybir.dt.float32
    f32r = mybir.dt.float32r
    bf16 = mybir.dt.bfloat16
    i32 = mybir.dt.int32
    u32 = mybir.dt.uint32
    AF = mybir.ActivationFunctionType
    ALU = mybir.AluOpType

    nc = bacc.Bacc(
        "TRN2",
        target_bir_lowering=False,
        debug=False,
        enable_asserts=True,
        num_devices=NCORES,
    )
    rqh_ext = nc.declare_dram_parameter("rq_host", [128, 32], f32, isOutput=False)
    ref_ext = nc.declare_dram_parameter("ref", [SEQ, BL, D], f32, isOutput=False)
    mask_ext = nc.declare_dram_parameter("mask", [SEQ, BL], f32, isOutput=False)
    wq_ext = nc.declare_dram_parameter("w_q", [D, D], f32, isOutput=False)
    wr_ext = nc.declare_dram_parameter("w_ref", [D, D], f32, isOutput=False)
    v_ext = nc.declare_dram_parameter("v", [D, 1], f32, isOutput=False)
    bdw_ext = nc.declare_dram_parameter("bdw_bf", [128, 128], bf16, isOutput=False)
    bdwq_ext = nc.declare_dram_parameter("bdwq", [128, 128], f32, isOutput=False)
    lhs2_ext = nc.declare_dram_parameter(
        "lhs2v_bf", [128, NG * 128], bf16, isOutput=False
    )
    prob_ext = nc.declare_dram_parameter("prob", [BL, SEQ], f32, isOutput=True)
    samp_ext = nc.declare_dram_parameter("sample", [BL], i32, isOutput=True)
    cand_ext = nc.declare_dram_parameter("cand", [BL, 8], i32, isOutput=True)

    with tile.TileContext(nc) as tc, ExitStack() as ctx:
        const_pool = ctx.enter_context(tc.tile_pool(name="const", bufs=1))
        nat_pool = ctx.enter_context(tc.tile_pool(name="nat", bufs=16))
        rhs_pool = ctx.enter_context(tc.tile_pool(name="rhs", bufs=6))
        tanh_pool = ctx.enter_context(tc.tile_pool(name="tanh", bufs=4))
        u_pool = ctx.enter_context(tc.tile_pool(name="u", bufs=1))
        prob_pool = ctx.enter_context(tc.tile_pool(name="prob", bufs=1))
        mt_pool = ctx.enter_context(tc.tile_pool(name="mt", bufs=2))
        stat_pool = ctx.enter_context(tc.tile_pool(name="stat", bufs=1))
        ps_pool = ctx.enter_context(tc.tile_pool(name="ps", bufs=1, space="PSUM"))

        # Persistent PSUM tiles, ping-ponged manually: pooled PSUM slots would
        # add release waits that exceed the matmul 1-wait codegen limit.
        pt_ps = [
            ps_pool.tile([128, SCHUNK], bf16, tag=f"pt{i}", name=f"pt{i}")
            for i in range(3)
        ]
        p1_ps = [
            ps_pool.tile([128, SCHUNK], f32, tag=f"p1{i}", name=f"p1{i}")
            for i in range(3)
        ]
        pu_ps = [
            ps_pool.tile([128, SCHUNK], f32, tag=f"pu{i}", name=f"pu{i}")
            for i in range(2)
        ]


        # ---------- constants ----------
        ident_f = const_pool.tile([128, 128], f32, tag="ident_f")
        masks.make_identity(nc, ident_f[:])
        ident = const_pool.tile([128, 128], bf16, tag="ident")
        nc.vector.tensor_copy(ident[:], ident_f[:])

        bdW = const_pool.tile([128, 128], bf16, tag="bdW")
        nc.scalar.dma_start(out=bdW[:], in_=bdw_ext[:, :])
        lhs2 = const_pool.tile([128, NG * 128], bf16, tag="lhs2")
        nc.scalar.dma_start(out=lhs2[:], in_=lhs2_ext[:, :])

        bdWq_st = const_pool.tile([128, 128], f32, tag="bdWq_st")
        nc.scalar.dma_start(out=bdWq_st[:], in_=bdwq_ext[:, :])
        bdWq = const_pool.tile([128, 128], f32r, tag="bdWq")
        nc.vector.tensor_copy(bdWq[:], bdWq_st[:])

        # query features in interleaved layout: rq[u*16+d, G] = query[8G+u, d]
        rq_st = const_pool.tile([128, 32], f32, tag="rq_st")
        nc.scalar.dma_start(out=rq_st[:], in_=rqh_ext[:, :])
        rq = const_pool.tile([128, 32], f32r, tag="rq")
        nc.vector.tensor_copy(rq[:], rq_st[:])
        psq = p1_ps[0]
        nc.tensor.matmul(psq[:, 0:32], bdWq[:], rq[:], start=True, stop=True)
        qf = const_pool.tile([128, 32], f32, tag="qf")
        nc.vector.tensor_copy(qf[:], psq[:, 0:32])

        # mask, natural layout: [128 p, 16 j, 256 b] with s = j*128 + p
        mask_nat = const_pool.tile([128, 16, BL], f32, tag="mask")
        nc.sync.dma_start(
            out=mask_nat[:], in_=mask_ext.rearrange("(j p) b -> p j b", p=128)
        )

        samp_sb = const_pool.tile([128, NH], i32, tag="samp")
        negE10 = const_pool.tile([128, 1], f32, tag="negE10")
        nc.gpsimd.memset(negE10[:], -1e10)


        # ---------- streaming ref loads (natural layout) ----------
        nat_tiles = []
        for j in range(16):
            t = nat_pool.tile([128, BL * D], bf16, tag="nat")
            nc.gpsimd.dma_start(
                out=t[:],
                in_=ref_ext[j * 128 : (j + 1) * 128].rearrange("p b d -> p (b d)"),
            )
            nat_tiles.append(t)

        U_sb = [u_pool.tile([128, SEQ], f32, tag=f"U{h}", name=f"U{h}") for h in range(NH)]

        # ---------- main loop ----------
        # Software-pipelined by 2 groups: transposes for group gi+2 are
        # emitted before matmul1 of group gi, so the PE never waits on the
        # DVE PSUM->SBUF copy.
        NU = NSC * NH
        TOT = NU * NG

        def unit_of(gi):
            return gi // NG

        def unit_ch(u):
            return u % NSC, u // NSC

        def emit_transposes(gi):
            u = unit_of(gi)
            c, h = unit_ch(u)
            g = gi % NG
            pt = pt_ps[gi % 3]
            col0 = (h * 128 + 8 * g) * 16
            for jj in range(4):
                blk = nat_tiles[4 * c + jj][:, col0 : col0 + 128]
                nc.tensor.transpose(
                    pt[:, jj * 128 : (jj + 1) * 128], blk, ident[:]
                )
            rhs = rhs_pool.tile([128, SCHUNK], bf16, tag="rhs", name="rhs")
            nc.vector.tensor_copy(rhs[:], pt[:])
            return rhs

        def emit_mm1_tanh(gi, rhs):
            u = unit_of(gi)
            c, h = unit_ch(u)
            g = gi % NG
            p1 = p1_ps[gi % 3]
            nc.tensor.matmul(p1[:], bdW[:], rhs[:], start=True, stop=True)
            th = tanh_pool.tile([128, SCHUNK], bf16, tag="tanh", name="th")
            G = h * 16 + g
            nc.scalar.activation(
                th[:], p1[:], AF.Tanh, bias=qf[:, G : G + 1], scale=1.0
            )
            return th

        def emit_mm2(gi, th):
            u = unit_of(gi)
            g = gi % NG
            pu = pu_ps[u % 2]
            nc.tensor.matmul(
                pu[:],
                lhs2[:, g * 128 : (g + 1) * 128],
                th[:],
                start=(g == 0),
                stop=(g == NG - 1),
            )

        def emit_epilogue(u):
            c, h = unit_ch(u)
            pu = pu_ps[u % 2]
            uc = U_sb[h][:, c * SCHUNK : (c + 1) * SCHUNK]
            nc.scalar.activation(uc, pu[:], AF.Tanh)
            pm = p1_ps[u % 2]
            for jj in range(4):
                mblk = mask_nat[:, 4 * c + jj, h * 128 : (h + 1) * 128]
                nc.tensor.transpose(
                    pm[:, jj * 128 : (jj + 1) * 128], mblk, ident_f[:]
                )
            mt = mt_pool.tile([128, SCHUNK], f32, tag="mt", name="mt")
            nc.vector.tensor_scalar(
                mt[:], pm[:], 1e10, -1e10, ALU.mult, ALU.add
            )
            nc.gpsimd.tensor_tensor(uc, uc, mt[:], op=ALU.add)

        # ---------- softmax + argmax (emitted right after each half) ----
        def emit_softmax(h):
            U = U_sb[h]
            mx = stat_pool.tile([128, 1], f32, tag=f"mx{h}", name=f"mx{h}")
            nc.vector.tensor_reduce(
                mx[:], U[:], axis=mybir.AxisListType.X, op=ALU.max
            )
            nbias = stat_pool.tile([128, 1], f32, tag=f"nb{h}", name=f"nb{h}")
            nc.scalar.mul(nbias[:], mx[:], -CSCALE)
            P = prob_pool.tile([128, SEQ], f32, tag=f"P{h}", name=f"P{h}")
            ssum = stat_pool.tile([128, 1], f32, tag=f"ss{h}", name=f"ss{h}")
            nc.scalar.activation(
                P[:], U[:], AF.Exp, bias=nbias[:], scale=CSCALE, accum_out=ssum[:]
            )
            rin = stat_pool.tile([128, 1], f32, tag=f"ri{h}", name=f"ri{h}")
            nc.vector.reciprocal(rin[:], ssum[:])
            nc.vector.tensor_scalar_mul(P[:], P[:], rin[:])
            nc.sync.dma_start(out=prob_ext[h * 128 : (h + 1) * 128, :], in_=P[:])
            mx8 = stat_pool.tile([128, 8], f32, tag=f"m8{h}", name=f"m8{h}")
            ix8 = stat_pool.tile([128, 8], u32, tag=f"i8{h}", name=f"i8{h}")
            nc.vector.max(mx8[:], P[:])
            nc.vector.max_index(ix8[:], mx8[:], P[:])
            nc.vector.tensor_copy(samp_sb[:, h : h + 1], ix8[:, 0:1])
            ci32 = stat_pool.tile([128, 8], i32, tag=f"c8{h}", name=f"ci32_{h}")
            nc.vector.tensor_copy(ci32[:], ix8[:])
            nc.sync.dma_start(
                out=cand_ext[h * 128 : (h + 1) * 128, :], in_=ci32[:]
            )

        rhs_q = {}
        th_q = {}
        rhs_q[0] = emit_transposes(0)
        rhs_q[1] = emit_transposes(1)
        for gi in range(TOT):
            u = unit_of(gi)
            if gi + 2 < TOT:
                rhs_q[gi + 2] = emit_transposes(gi + 2)
            th_q[gi] = emit_mm1_tanh(gi, rhs_q.pop(gi))
            emit_mm2(gi, th_q.pop(gi))
            if gi % NG == NG - 1:
                emit_epilogue(u)
                if u == NSC - 1:
                    emit_softmax(0)
        emit_softmax(1)

        nc.sync.dma_start(
            out=samp_ext.rearrange("(h p) -> p h", p=128), in_=samp_sb[:]
        )

    return nc


def _get_nc():
    if "nc" not in _CACHE:
        nc = _build_bass()
        nc.finalize()  # run bacc passes (wait splitting, reg alloc, codegen)
        _CACHE["nc"] = nc
    return _CACHE["nc"]


def run(query, ref, mask, w_q, w_ref, v, trace=False):
    from concourse.bass_utils import run_bass_kernel_spmd

    query = np.asarray(query, dtype=np.float32)
    ref = np.asarray(ref, dtype=np.float32)
    mask = np.asarray(mask, dtype=np.float32)
    w_q = np.ascontiguousarray(np.asarray(w_q, dtype=np.float32))
    w_ref = np.ascontiguousarray(np.asarray(w_ref, dtype=np.float32))
    v = np.ascontiguousarray(np.asarray(v, dtype=np.float32))

    import ml_dtypes
    bf = ml_dtypes.bfloat16
    bdW_np = np.zeros((128, 128), np.float32)
    bdWq_np = np.zeros((128, 128), np.float32)
    lhs2_np = np.zeros((128, NG * 128), np.float32)
    for u in range(8):
        bdW_np[16 * u : 16 * u + 16, 16 * u : 16 * u + 16] = w_ref
        bdWq_np[16 * u : 16 * u + 16, 16 * u : 16 * u + 16] = w_q
    for g in range(NG):
        for u in range(8):
            lhs2_np[16 * u : 16 * u + 16, g * 128 + 8 * g + u] = v[:, 0]

    # rq_host[u*16+d, G] = query[8G+u, d] per core (prearranged, contiguous)
    nc = _get_nc()
    in_maps = []
    for c in range(NCORES):
        sl = slice(c * BL, (c + 1) * BL)
        in_maps.append(
            {
                "rq_host": np.ascontiguousarray(
                    query[0, sl, :]
                    .reshape(32, 8, 16)
                    .transpose(1, 2, 0)
                    .reshape(128, 32)
                ),
                "ref": np.ascontiguousarray(ref[:, sl, :]),
                "mask": np.ascontiguousarray(mask[:, sl]),
                "w_q": w_q,
                "w_ref": w_ref,
                "v": v,
                "bdw_bf": bdW_np.astype(bf),
                "bdwq": bdWq_np,
                "lhs2v_bf": lhs2_np.astype(bf),
            }
        )
    res = run_bass_kernel_spmd(
        nc, in_maps, core_ids=list(range(NCORES)), trace=trace
    )
    prob = np.concatenate(
        [np.asarray(res.results[i]["prob"]) for i in range(NCORES)], axis=0
    )
    cand = np.concatenate(
        [np.asarray(res.results[i]["cand"]) for i in range(NCORES)], axis=0
    ).astype(np.int64)

    # Exact rescoring of the device's top-8 candidates per row (f64): the
    # device search uses f32r matmuls (~5e-4), which can flip near-tied
    # argmax results; an 8-way exact comparison on the host fixes that.
    b_idx = np.arange(B)[:, None]
    r = ref[cand, b_idx, :].astype(np.float64)  # (B, 8, D)
    qf64 = query[0].astype(np.float64) @ w_q.astype(np.float64)  # (B, D)
    y = np.tanh(r @ w_ref.astype(np.float64) + qf64[:, None, :])
    u = np.tanh(y @ v.astype(np.float64))[..., 0]  # (B, 8)
    u = u + (1.0 - mask[cand, b_idx].astype(np.float64)) * -1e10
    sample = cand[np.arange(B), np.argmax(u, axis=1)].astype(np.int32)
    return prob, sample, res


def kernel(query, ref, mask, w_q, w_ref, v):
    prob, sample, _ = run(query, ref, mask, w_q, w_ref, v, trace=False)
    return prob, sample


# revision 34
# speedup vs baseline: 1.0899x; 1.0899x over previous
"""Bahdanau-attention kernel for Trainium2, data-parallel over batch on 8 NeuronCores.

Computes, per the reference:
    qf   = (query @ w_q)[0]                       # (B, 16)
    U    = tanh(ref @ w_ref + qf) @ v             # (SEQ, B)
    U    = tanh(U)
    U   += (1 - mask) * -1e10
    prob = softmax(U * 10, axis=0).T              # (B, SEQ)
    samp = argmax(prob, axis=1)                   # (B,)

Sharding: batch axis split 8 ways (256 batches/core); softmax over SEQ is local.

Per-core dataflow (B_local=256 = 2 halves x 128 partitions; groups of 8 batches
pack the 16-dim feature axis into 128 partitions as p = u*16+d):
  1. DMA ref in natural layout [128 s-rows, 256*16 free] (contiguous 16KB rows).
  2. PE transpose 128x128 blocks (one block = 8 batches x 16 dims) -> PSUM.
  3. DVE/ACT copy PSUM -> SBUF (PE cannot read PSUM).
  4. matmul vs block-diag(w_ref) in float32r (full-rate fp32) -> Y^T in PSUM.
  5. ScalarE tanh with per-partition bias = query-feature column -> SBUF.
  6. matmul vs per-group block-diag(v), accumulating U [128 b, 512 s] in PSUM.
  7. ScalarE tanh -> U in SBUF; add (1-mask^T)*-1e10 (mask^T via PE transpose).
  8. Softmax along free axis: reduce_max, Exp with fused bias/scale/accum-sum,
     reciprocal, scale; argmax via DVE max/max_index. DMA prob + sample out.
"""

import sys

import numpy as np

if "/opt/trn_rl_repo" not in sys.path:
    sys.path.insert(0, "/opt/trn_rl_repo")

SEQ = 2048
B = 2048
D = 16
NCORES = 8
BL = B // NCORES  # 256 batches per core
NH = 2  # halves of 128 batches
SCHUNK = 512  # seq positions per chunk
NSC = SEQ // SCHUNK  # 4
NG = 16  # groups of 8 batches per half
CSCALE = 10.0

_CACHE = {}


def _build_bass():
    import concourse.bass as bass
    import concourse.mybir as mybir
    import concourse.tile as tile
    from concourse import bacc, masks
    from contextlib import ExitStack

    f32 = mybir.dt.float32
    f32r = mybir.dt.float32r
    bf16 = mybir.dt.bfloat16
    i32 = mybir.dt.int32
    u32 = mybir.dt.uint32
    AF = mybir.ActivationFunctionType
    ALU = mybir.AluOpType

    nc = bacc.Bacc(
        "TRN2",
        target_bir_lowering=False,
        debug=False,
        enable_asserts=True,
        num_devices=NCORES,
    )
    rqh_ext = nc.declare_dram_parameter("rq_host", [128, 32], f32, isOutput=False)
    ref_ext = nc.declare_dram_parameter("ref", [SEQ, BL, D], f32, isOutput=False)
    mask_ext = nc.declare_dram_parameter("mask", [SEQ, BL], f32, isOutput=False)
    wq_ext = nc.declare_dram_parameter("w_q", [D, D], f32, isOutput=False)
    wr_ext = nc.declare_dram_parameter("w_ref", [D, D], f32, isOutput=False)
    v_ext = nc.declare_dram_parameter("v", [D, 1], f32, isOutput=False)
    bdw_ext = nc.declare_dram_parameter("bdw_bf", [128, 128], bf16, isOutput=False)
    bdwq_ext = nc.declare_dram_parameter("bdwq", [128, 128], f32, isOutput=False)
    lhs2_ext = nc.declare_dram_parameter(
        "lhs2v_bf", [128, NG * 128], bf16, isOutput=False
    )
    prob_ext = nc.declare_dram_parameter("prob", [BL, SEQ], f32, isOutput=True)
    samp_ext = nc.declare_dram_parameter("sample", [BL], i32, isOutput=True)
    cand_ext = nc.declare_dram_parameter("cand", [BL, 8], i32, isOutput=True)

    with tile.TileContext(nc) as tc, ExitStack() as ctx:
        const_pool = ctx.enter_context(tc.tile_pool(name="const", bufs=1))
        nat_pool = ctx.enter_context(tc.tile_pool(name="nat", bufs=10))
        rhs_pool = ctx.enter_context(tc.tile_pool(name="rhs", bufs=6))
        tanh_pool = ctx.enter_context(tc.tile_pool(name="tanh", bufs=4))
        u_pool = ctx.enter_context(tc.tile_pool(name="u", bufs=1))
        prob_pool = ctx.enter_context(tc.tile_pool(name="prob", bufs=1))
        mt_pool = ctx.enter_context(tc.tile_pool(name="mt", bufs=2))
        stat_pool = ctx.enter_context(tc.tile_pool(name="stat", bufs=1))
        ps_pool = ctx.enter_context(tc.tile_pool(name="ps", bufs=1, space="PSUM"))

        # Persistent PSUM tiles, ping-ponged manually: pooled PSUM slots would
        # add release waits that exceed the matmul 1-wait codegen limit.
        pt_ps = [
            ps_pool.tile([128, SCHUNK], bf16, tag=f"pt{i}", name=f"pt{i}")
            for i in range(3)
        ]
        p1_ps = [
            ps_pool.tile([128, SCHUNK], f32, tag=f"p1{i}", name=f"p1{i}")
            for i in range(3)
        ]
        pu_ps = [
            ps_pool.tile([128, SCHUNK], f32, tag=f"pu{i}", name=f"pu{i}")
            for i in range(2)
        ]


        # ---------- constants ----------
        ident_f = const_pool.tile([128, 128], f32, tag="ident_f")
        masks.make_identity(nc, ident_f[:])
        ident = const_pool.tile([128, 128], bf16, tag="ident")
        nc.vector.tensor_copy(ident[:], ident_f[:])

        bdW = const_pool.tile([128, 128], bf16, tag="bdW")
        nc.scalar.dma_start(out=bdW[:], in_=bdw_ext[:, :])
        lhs2 = const_pool.tile([128, NG * 128], bf16, tag="lhs2")
        nc.scalar.dma_start(out=lhs2[:], in_=lhs2_ext[:, :])

        bdWq_st = const_pool.tile([128, 128], f32, tag="bdWq_st")
        nc.scalar.dma_start(out=bdWq_st[:], in_=bdwq_ext[:, :])
        bdWq = const_pool.tile([128, 128], f32r, tag="bdWq")
        nc.vector.tensor_copy(bdWq[:], bdWq_st[:])

        # query features in interleaved layout: rq[u*16+d, G] = query[8G+u, d]
        rq_st = const_pool.tile([128, 32], f32, tag="rq_st")
        nc.scalar.dma_start(out=rq_st[:], in_=rqh_ext[:, :])
        rq = const_pool.tile([128, 32], f32r, tag="rq")
        nc.vector.tensor_copy(rq[:], rq_st[:])
        psq = p1_ps[0]
        nc.tensor.matmul(psq[:, 0:32], bdWq[:], rq[:], start=True, stop=True)
        qf = const_pool.tile([128, 32], f32, tag="qf")
        nc.vector.tensor_copy(qf[:], psq[:, 0:32])

        # mask, natural layout: [128 p, 16 j, 256 b] with s = j*128 + p
        mask_nat = const_pool.tile([128, 16, BL], f32, tag="mask")
        nc.sync.dma_start(
            out=mask_nat[:], in_=mask_ext.rearrange("(j p) b -> p j b", p=128)
        )

        samp_sb = const_pool.tile([128, NH], i32, tag="samp")
        negE10 = const_pool.tile([128, 1], f32, tag="negE10")
        nc.gpsimd.memset(negE10[:], -1e10)


        # ---------- streaming ref loads (natural layout) ----------
        nat_tiles = []
        for j in range(16):
            t = nat_pool.tile([128, BL * D], bf16, tag="nat")
            nc.gpsimd.dma_start(
                out=t[:],
                in_=ref_ext[j * 128 : (j + 1) * 128].rearrange("p b d -> p (b d)"),
            )
            nat_tiles.append(t)

        U_sb = [u_pool.tile([128, SEQ], f32, tag=f"U{h}", name=f"U{h}") for h in range(NH)]

        # ---------- main loop ----------
        # Software-pipelined by 2 groups: transposes for group gi+2 are
        # emitted before matmul1 of group gi, so the PE never waits on the
        # DVE PSUM->SBUF copy.
        NU = NSC * NH
        TOT = NU * NG

        def unit_of(gi):
            return gi // NG

        def emit_transposes(gi):
            u = unit_of(gi)
            c, h = u // NH, u % NH
            g = gi % NG
            pt = pt_ps[gi % 3]
            col0 = (h * 128 + 8 * g) * 16
            for jj in range(4):
                blk = nat_tiles[4 * c + jj][:, col0 : col0 + 128]
                nc.tensor.transpose(
                    pt[:, jj * 128 : (jj + 1) * 128], blk, ident[:]
                )
            rhs = rhs_pool.tile([128, SCHUNK], bf16, tag="rhs", name="rhs")
            nc.vector.tensor_copy(rhs[:], pt[:])
            return rhs

        def emit_mm1_tanh(gi, rhs):
            u = unit_of(gi)
            c, h = u // NH, u % NH
            g = gi % NG
            p1 = p1_ps[gi % 3]
            nc.tensor.matmul(p1[:], bdW[:], rhs[:], start=True, stop=True)
            th = tanh_pool.tile([128, SCHUNK], bf16, tag="tanh", name="th")
            G = h * 16 + g
            nc.scalar.activation(
                th[:], p1[:], AF.Tanh, bias=qf[:, G : G + 1], scale=1.0
            )
            return th

        def emit_mm2(gi, th):
            u = unit_of(gi)
            g = gi % NG
            pu = pu_ps[u % 2]
            nc.tensor.matmul(
                pu[:],
                lhs2[:, g * 128 : (g + 1) * 128],
                th[:],
                start=(g == 0),
                stop=(g == NG - 1),
            )

        def emit_epilogue(u):
            c, h = u // NH, u % NH
            pu = pu_ps[u % 2]
            uc = U_sb[h][:, c * SCHUNK : (c + 1) * SCHUNK]
            nc.scalar.activation(uc, pu[:], AF.Tanh)
            pm = p1_ps[u % 2]
            for jj in range(4):
                mblk = mask_nat[:, 4 * c + jj, h * 128 : (h + 1) * 128]
                nc.tensor.transpose(
                    pm[:, jj * 128 : (jj + 1) * 128], mblk, ident_f[:]
                )
            mt = mt_pool.tile([128, SCHUNK], f32, tag="mt", name="mt")
            nc.vector.tensor_scalar(
                mt[:], pm[:], 1e10, -1e10, ALU.mult, ALU.add
            )
            nc.gpsimd.tensor_tensor(uc, uc, mt[:], op=ALU.add)

        rhs_q = {}
        th_q = {}
        rhs_q[0] = emit_transposes(0)
        rhs_q[1] = emit_transposes(1)
        for gi in range(TOT):
            u = unit_of(gi)
            if gi + 2 < TOT:
                rhs_q[gi + 2] = emit_transposes(gi + 2)
            th_q[gi] = emit_mm1_tanh(gi, rhs_q.pop(gi))
            emit_mm2(gi, th_q.pop(gi))
            if gi % NG == NG - 1:
                emit_epilogue(u)

        # ---------- softmax + argmax per half ----------
        for h in range(NH):
            U = U_sb[h]
            mx = stat_pool.tile([128, 1], f32, tag=f"mx{h}")
            nc.vector.tensor_reduce(
                mx[:], U[:], axis=mybir.AxisListType.X, op=ALU.max
            )
            nbias = stat_pool.tile([128, 1], f32, tag=f"nb{h}")
            nc.scalar.mul(nbias[:], mx[:], -CSCALE)
            P = prob_pool.tile([128, SEQ], f32, tag=f"P{h}")
            ssum = stat_pool.tile([128, 1], f32, tag=f"ss{h}")
            nc.scalar.activation(
                P[:], U[:], AF.Exp, bias=nbias[:], scale=CSCALE, accum_out=ssum[:]
            )
            rin = stat_pool.tile([128, 1], f32, tag=f"ri{h}")
            nc.vector.reciprocal(rin[:], ssum[:])
            nc.vector.tensor_scalar_mul(P[:], P[:], rin[:])
            nc.sync.dma_start(out=prob_ext[h * 128 : (h + 1) * 128, :], in_=P[:])
            mx8 = stat_pool.tile([128, 8], f32, tag=f"m8{h}")
            ix8 = stat_pool.tile([128, 8], u32, tag=f"i8{h}")
            nc.vector.max(mx8[:], P[:])
            nc.vector.max_index(ix8[:], mx8[:], P[:])
            nc.vector.tensor_copy(samp_sb[:, h : h + 1], ix8[:, 0:1])
            ci32 = stat_pool.tile([128, 8], i32, tag=f"c8{h}", name=f"ci32_{h}")
            nc.vector.tensor_copy(ci32[:], ix8[:])
            nc.sync.dma_start(
                out=cand_ext[h * 128 : (h + 1) * 128, :], in_=ci32[:]
            )

        nc.sync.dma_start(
            out=samp_ext.rearrange("(h p) -> p h", p=128), in_=samp_sb[:]
        )

    return nc


def _get_nc():
    if "nc" not in _CACHE:
        nc = _build_bass()
        nc.finalize()  # run bacc passes (wait splitting, reg alloc, codegen)
        _CACHE["nc"] = nc
    return _CACHE["nc"]


def run(query, ref, mask, w_q, w_ref, v, trace=False):
    from concourse.bass_utils import run_bass_kernel_spmd

    query = np.asarray(query, dtype=np.float32)
    ref = np.asarray(ref, dtype=np.float32)
    mask = np.asarray(mask, dtype=np.float32)
    w_q = np.ascontiguousarray(np.asarray(w_q, dtype=np.float32))
    w_ref = np.ascontiguousarray(np.asarray(w_ref, dtype=np.float32))
    v = np.ascontiguousarray(np.asarray(v, dtype=np.float32))

    import ml_dtypes
    bf = ml_dtypes.bfloat16
    bdW_np = np.zeros((128, 128), np.float32)
    bdWq_np = np.zeros((128, 128), np.float32)
    lhs2_np = np.zeros((128, NG * 128), np.float32)
    for u in range(8):
        bdW_np[16 * u : 16 * u + 16, 16 * u : 16 * u + 16] = w_ref
        bdWq_np[16 * u : 16 * u + 16, 16 * u : 16 * u + 16] = w_q
    for g in range(NG):
        for u in range(8):
            lhs2_np[16 * u : 16 * u + 16, g * 128 + 8 * g + u] = v[:, 0]

    # rq_host[u*16+d, G] = query[8G+u, d] per core (prearranged, contiguous)
    nc = _get_nc()
    in_maps = []
    for c in range(NCORES):
        sl = slice(c * BL, (c + 1) * BL)
        in_maps.append(
            {
                "rq_host": np.ascontiguousarray(
                    query[0, sl, :]
                    .reshape(32, 8, 16)
                    .transpose(1, 2, 0)
                    .reshape(128, 32)
                ),
                "ref": np.ascontiguousarray(ref[:, sl, :]),
                "mask": np.ascontiguousarray(mask[:, sl]),
                "w_q": w_q,
                "w_ref": w_ref,
                "v": v,
                "bdw_bf": bdW_np.astype(bf),
                "bdwq": bdWq_np,
                "lhs2v_bf": lhs2_np.astype(bf),
            }
        )
    res = run_bass_kernel_spmd(
        nc, in_maps, core_ids=list(range(NCORES)), trace=trace
    )
    prob = np.concatenate(
        [np.asarray(res.results[i]["prob"]) for i in range(NCORES)], axis=0
    )
    cand = np.concatenate(
        [np.asarray(res.results[i]["cand"]) for i in range(NCORES)], axis=0
    ).astype(np.int64)

    # Exact rescoring of the device's top-8 candidates per row (f64): the
    # device search uses f32r matmuls (~5e-4), which can flip near-tied
    # argmax results; an 8-way exact comparison on the host fixes that.
    b_idx = np.arange(B)[:, None]
    r = ref[cand, b_idx, :].astype(np.float64)  # (B, 8, D)
    qf64 = query[0].astype(np.float64) @ w_q.astype(np.float64)  # (B, D)
    y = np.tanh(r @ w_ref.astype(np.float64) + qf64[:, None, :])
    u = np.tanh(y @ v.astype(np.float64))[..., 0]  # (B, 8)
    u = u + (1.0 - mask[cand, b_idx].astype(np.float64)) * -1e10
    sample = cand[np.arange(B), np.argmax(u, axis=1)].astype(np.int32)
    return prob, sample, res


def kernel(query, ref, mask, w_q, w_ref, v):
    prob, sample, _ = run(query, ref, mask, w_q, w_ref, v, trace=False)
    return prob, sample
